# revision 1
# baseline (speedup 1.0000x reference)
"""GEAR quantized-KV Llama attention decode step on 8 trn2 NeuronCores.

Sharding: tensor-parallel over heads (4 heads/core x 8 cores), all batches on
every core; each core computes a partial wo-product, summed on host.
"""
import os
import sys
import math

sys.path.insert(0, "/opt/trn_rl_repo")
import numpy as np
from contextlib import ExitStack

import concourse.bass as bass
import concourse.mybir as mybir
import concourse.tile as tile
from concourse import bacc, bass_isa
from concourse.bass_utils import run_bass_kernel_spmd
from concourse.masks import make_identity

B, H, D, HID = 4, 32, 128, 4096
SQ, SF, QL = 4096, 63, 1
GS, RANK = 64, 4
THETA = 10000.0
NCORES = 8
HPC = H // NCORES          # heads per core = 4
NCH = SQ // 128            # 32 s-chunks
G = SQ // GS               # 64 groups along seq (K side)
FD = D // GS               # 2 groups along head_dim (V side)
SFP = SF + 1               # 64 full-precision keys incl the new token
DT = mybir.dt
ISQD = 1.0 / math.sqrt(D)

_CACHE = {}


def _build():
    nc = bacc.Bacc("TRN2", target_bir_lowering=False)
    f32, bf16, i32 = DT.float32, DT.bfloat16, DT.int32

    hidT = nc.declare_dram_parameter("hidT", [HID, B], f32, isOutput=False)
    cost = nc.declare_dram_parameter("cost", [B, HPC * D], f32, isOutput=False)
    sint = nc.declare_dram_parameter("sint", [B, HPC * D], f32, isOutput=False)
    wT = {w: nc.declare_dram_parameter(w, [HID, HPC * D], f32, isOutput=False) for w in ("wqT", "wkT", "wvT")}
    woT = nc.declare_dram_parameter("woT", [HPC * D, HID], f32, isOutput=False)
    kcode = nc.declare_dram_parameter("kcode", [B, HPC, D, SQ], i32, isOutput=False)
    kscale = nc.declare_dram_parameter("kscale", [B, HPC, D, G], f32, isOutput=False)
    kmn = nc.declare_dram_parameter("kmn", [B, HPC, D, G], f32, isOutput=False)
    kfT = nc.declare_dram_parameter("kfT", [B, HPC, D, SF], f32, isOutput=False)
    kp = nc.declare_dram_parameter("kp", [B, HPC, 128, NCH, RANK], f32, isOutput=False)
    keyq = nc.declare_dram_parameter("keyq", [B, HPC, D, RANK], f32, isOutput=False)
    vcode = nc.declare_dram_parameter("vcode", [B, HPC, SQ, D], i32, isOutput=False)
    vscT = nc.declare_dram_parameter("vscT", [B, HPC, 128, NCH, FD], f32, isOutput=False)
    vmnT = nc.declare_dram_parameter("vmnT", [B, HPC, 128, NCH, FD], f32, isOutput=False)
    vqT = nc.declare_dram_parameter("vqT", [B, HPC, 128, NCH, RANK], f32, isOutput=False)
    vpT = nc.declare_dram_parameter("vpT", [B, HPC, 7, D], f32, isOutput=False)  # rows 0-2 zero
    vfr = nc.declare_dram_parameter("vfr", [B, HPC, SF, D], f32, isOutput=False)
    out = nc.declare_dram_parameter("out", [B, HID], f32, isOutput=True)

    AO = mybir.AluOpType
    AF = mybir.ActivationFunctionType

    with tile.TileContext(nc) as tc, ExitStack() as ctx:
        const = ctx.enter_context(tc.tile_pool(name="const", bufs=1))
        pw = ctx.enter_context(tc.tile_pool(name="pw", bufs=2))
        psC = ctx.enter_context(tc.tile_pool(name="psC", bufs=2, space="PSUM"))
        psW = ctx.enter_context(tc.tile_pool(name="psW", bufs=1, space="PSUM"))
        ictx = ctx.enter_context(ExitStack())
        psml = ictx.enter_context(tc.tile_pool(name="psml", bufs=3))
        pkc = ictx.enter_context(tc.tile_pool(name="pkc", bufs=2))
        pvt = ictx.enter_context(tc.tile_pool(name="pvt", bufs=2))
        psA = ictx.enter_context(tc.tile_pool(name="psA", bufs=2, space="PSUM"))
        psB = ictx.enter_context(tc.tile_pool(name="psB", bufs=2, space="PSUM"))

        # ---- constants ----
        id4 = const.tile([4, 4], f32)
        make_identity(nc, id4[:])
        id16 = const.tile([16, 16], f32)
        make_identity(nc, id16[:], nomemset=False)
        hid_sb = const.tile([128, HID // 128, B], f32)
        nc.sync.dma_start(out=hid_sb[:], in_=hidT[:].rearrange("(c p) b -> p c b", p=128))
        cos_sb = const.tile([B, HPC * D], f32)
        nc.sync.dma_start(out=cos_sb[:], in_=cost[:])
        sin_sb = const.tile([B, HPC * D], f32)
        nc.sync.dma_start(out=sin_sb[:], in_=sint[:])

        # ---- projections: psum[b, 512] = sum_c hidT_c^T @ wT_c ----
        proj = {}
        for wname in ("wqT", "wkT", "wvT"):
            pps = psC.tile([B, HPC * D], f32, tag="misc")
            for blk in range(4):
                slab = pw.tile([128, 8, HPC * D], f32, tag="wslab")
                nc.sync.dma_start(
                    out=slab[:],
                    in_=wT[wname][:].rearrange("(c p) n -> p c n", p=128)[:, 8 * blk:8 * blk + 8, :],
                )
                for j in range(8):
                    c = 8 * blk + j
                    nc.tensor.matmul(pps[:], hid_sb[:, c, :], slab[:, j, :],
                                     start=(c == 0), stop=(c == 31))
            sb = const.tile([B, HPC * D], f32, tag=wname)
            nc.scalar.copy(sb[:], pps[:])
            proj[wname] = sb
        q_sb, k_sb, v_sb = proj["wqT"], proj["wkT"], proj["wvT"]

        # ---- RoPE on q and k (rows [B, HPC*D]) ----
        def rope(x_sb, tagp):
            rot = const.tile([B, HPC * D], f32, tag=tagp + "rot")
            xv = x_sb[:].rearrange("b (h two d) -> b h two d", two=2, d=64)
            rv = rot[:].rearrange("b (h two d) -> b h two d", two=2, d=64)
            nc.vector.tensor_scalar(rv[:, :, 0, :], xv[:, :, 1, :], -1.0, None, AO.mult)
            nc.vector.tensor_copy(rv[:, :, 1, :], xv[:, :, 0, :])
            nc.vector.tensor_tensor(rot[:], rot[:], sin_sb[:], AO.mult)
            ro = const.tile([B, HPC * D], f32, tag=tagp + "ro")
            nc.vector.tensor_tensor(ro[:], x_sb[:], cos_sb[:], AO.mult)
            nc.vector.tensor_tensor(ro[:], ro[:], rot[:], AO.add)
            return ro
        qro = rope(q_sb, "q")
        kro = rope(k_sb, "k")

        # per-head transposed columns: qscT [128, h, b] (scaled by 1/sqrt(D)), kT
        qscT = const.tile([128, HPC, B], f32)
        kT = const.tile([128, HPC, B], f32)
        for h in range(HPC):
            pq = psC.tile([128, B], f32, tag="misc")
            nc.tensor.transpose(pq[:], qro[0:B, h * D:(h + 1) * D], id4[:])
            nc.scalar.mul(qscT[:, h, :], pq[:], ISQD)
            pk = psC.tile([128, B], f32, tag="misc")
            nc.tensor.transpose(pk[:], kro[0:B, h * D:(h + 1) * D], id4[:])
            nc.scalar.copy(kT[:, h, :], pk[:])

        rows_sb = const.tile([16, 128], f32)
        woin_ps = psW.tile([128, 16], f32)

        # ---- per (b, h) attention ----
        for b in range(B):
            for h in range(HPC):
                idx = h * B + b
                qcol = qscT[:, h, b:b + 1]

                kc_bf = pkc.tile([128, SQ], bf16, tag="kc")
                nc.gpsimd.dma_start(out=kc_bf[:], in_=kcode[b, h])
                ksc = psml.tile([128, G], f32, tag="ksc")
                nc.sync.dma_start(out=ksc[:], in_=kscale[b, h])
                kmn_sb = psml.tile([128, G], f32, tag="kmn")
                nc.sync.dma_start(out=kmn_sb[:], in_=kmn[b, h])
                kfp = psml.tile([128, SFP], f32, tag="kfp")
                nc.sync.dma_start(out=kfp[:, 0:SF], in_=kfT[b, h])
                kp_sb = psml.tile([128, NCH, RANK], f32, tag="kp")
                nc.sync.dma_start(out=kp_sb[:], in_=kp[b, h])
                keyq_sb = psml.tile([128, RANK], f32, tag="keyq")
                nc.sync.dma_start(out=keyq_sb[:], in_=keyq[b, h])
                vt = pvt.tile([128, NCH, 131], bf16, tag="vt")
                nc.gpsimd.dma_start(out=vt[:, :, 0:128],
                                    in_=vcode[b, h].rearrange("(c p) d -> p c d", p=128))
                nc.gpsimd.dma_start(out=vt[:, :, 128:130], in_=vmnT[b, h])
                aw3 = psml.tile([128, NCH, 7], bf16, tag="aw3")
                nc.gpsimd.dma_start(out=aw3[:, :, 3:7], in_=vqT[b, h])
                vsc = psml.tile([128, NCH, FD], f32, tag="vsc")
                nc.sync.dma_start(out=vsc[:], in_=vscT[b, h])
                vpT_sb = psml.tile([7, D], f32, tag="vpT")
                nc.sync.dma_start(out=vpT_sb[:], in_=vpT[b, h])
                vf_sb = psml.tile([SFP, D], f32, tag="vf")
                nc.sync.dma_start(out=vf_sb[0:SF, :], in_=vfr[b, h])
                # new-token k/v into the full-precision blocks
                nc.vector.tensor_copy(kfp[:, SF:SFP], kT[:, h, b:b + 1])
                nc.sync.dma_start(out=vf_sb[SF:SFP, :], in_=v_sb[b:b + 1, h * D:(h + 1) * D])

                # quant K scores: psk[s, 2c + g'] over chunks
                qs = psml.tile([128, G], bf16, tag="qs")
                nc.vector.tensor_scalar(qs[:], ksc[:], qcol, None, AO.mult)
                psk = psA.tile([128, 2 * NCH], f32, tag="psk")
                for c in range(NCH):
                    nc.tensor.matmul(psk[:, 2 * c:2 * c + 2], kc_bf[:, c * 128:(c + 1) * 128],
                                     qs[:, 2 * c:2 * c + 2], start=True, stop=True)
                # misc: kf scores [0:64, 0:1]; qr row [0:1, 32:36]; mn bias row [0:1, 64:128]
                psm = psC.tile([128, 128], f32, tag="misc")
                nc.tensor.matmul(psm[0:SFP, 0:1], kfp[:], qcol, start=True, stop=True)
                nc.tensor.matmul(psm[0:1, 32:36], qcol, keyq_sb[:], start=True, stop=True)
                nc.tensor.matmul(psm[0:1, 64:128], qcol, kmn_sb[:], start=True, stop=True)

                qr_sb = psml.tile([1, RANK], f32, tag="qr")
                nc.scalar.copy(qr_sb[:], psm[0:1, 32:36])
                qrb = psml.tile([128, RANK], f32, tag="qrb")
                nc.gpsimd.partition_broadcast(qrb[:], qr_sb[:])
                bias_r = psml.tile([1, G], f32, tag="biasr")
                nc.scalar.copy(bias_r[:], psm[0:1, 64:128])
                bias_bc = psml.tile([128, G], f32, tag="biasbc")
                nc.gpsimd.partition_broadcast(bias_bc[:], bias_r[:])

                lrt = psml.tile([128, NCH, RANK], f32, tag="lrt")
                nc.vector.tensor_tensor(lrt[:], kp_sb[:],
                                        qrb[:, None, :].to_broadcast((128, NCH, RANK)), AO.mult)
                lr = psml.tile([128, NCH], f32, tag="lr")
                nc.vector.reduce_sum(lr[:], lrt[:], axis=mybir.AxisListType.X)

                att = psml.tile([128, NCH + 1], f32, tag="att")
                pskv = psk[:].rearrange("p (c two) -> p c two", two=2)
                bbv = bias_bc[:].rearrange("p (c two) -> p c two", two=2)
                nc.vector.tensor_tensor(att[0:64, 0:NCH], pskv[0:64, :, 0], lr[0:64, :], AO.add)
                nc.vector.tensor_tensor(att[0:64, 0:NCH], att[0:64, 0:NCH], bbv[0:64, :, 0], AO.add)
                nc.vector.tensor_tensor(att[64:128, 0:NCH], pskv[64:128, :, 1], lr[64:128, :], AO.add)
                nc.vector.tensor_tensor(att[64:128, 0:NCH], att[64:128, 0:NCH], bbv[64:128, :, 1], AO.add)
                nc.vector.memset(att[:, NCH:NCH + 1], -1e9)
                nc.vector.tensor_copy(att[0:SFP, NCH:NCH + 1], psm[0:SFP, 0:1])

                # softmax over all 128 x 33 entries
                m1 = psml.tile([128, 1], f32, tag="m1")
                nc.vector.reduce_max(m1[:], att[:], axis=mybir.AxisListType.X)
                mg = psml.tile([128, 1], f32, tag="mg")
                nc.gpsimd.partition_all_reduce(mg[:], m1[:], 128, bass_isa.ReduceOp.max)
                negm = psml.tile([128, 1], f32, tag="negm")
                nc.vector.tensor_scalar(negm[:], mg[:], -1.0, None, AO.mult)
                e = psml.tile([128, NCH + 1], bf16, tag="e")
                ssum = psml.tile([128, 1], f32, tag="ssum")
                nc.scalar.activation(e[:], att[:], AF.Exp, bias=negm[:, 0:1], scale=1.0,
                                     alpha=0.0, accum_out=ssum[:])
                sg = psml.tile([128, 1], f32, tag="sg")
                nc.gpsimd.partition_all_reduce(sg[:], ssum[:], 128, bass_isa.ReduceOp.add)
                recip = psml.tile([128, 1], f32, tag="recip")
                nc.vector.reciprocal(recip[:], sg[:])

                # build lhsT cols: 0 = aw, 1-2 = aw*vs, (3-6 = vq already)
                ev = e[:, 0:NCH, None]
                nc.vector.tensor_scalar(aw3[:, :, 0:1], ev, recip[:, 0:1], None, AO.mult)
                nc.vector.scalar_tensor_tensor(aw3[:, :, 1:3], ev.to_broadcast((128, NCH, FD)),
                                               recip[:, 0:1], vsc[:], AO.mult, AO.mult)
                nc.vector.tensor_scalar(vt[:, :, 130:131], ev, recip[:, 0:1], None, AO.mult)
                awf = psml.tile([SFP, 1], f32, tag="awf")
                nc.vector.tensor_scalar(awf[:], e[0:SFP, NCH:NCH + 1], recip[0:SFP, 0:1],
                                        None, AO.mult)

                psv = psB.tile([7, 131], f32, tag="psv")
                for c in range(NCH):
                    nc.tensor.matmul(psv[:], aw3[:, c, :], vt[:, c, :],
                                     start=(c == 0), stop=(c == NCH - 1))

                # mn scalars at partition 0; broadcast to partitions 1,2
                mn2 = psml.tile([3, FD], f32, tag="mn2")
                nc.scalar.copy(mn2[0:1, :], psv[0:1, 128:130])
                mn2b = psml.tile([3, FD], f32, tag="mn2b")
                nc.gpsimd.partition_broadcast(mn2b[:], mn2[0:1, :], channels=3)
                stage = psml.tile([3, 128], f32, tag="stage")
                nc.vector.tensor_scalar(stage[0:3, 0:64], psv[0:3, 0:64], mn2b[0:3, 0:1],
                                        None, AO.add)
                nc.vector.tensor_scalar(stage[0:3, 64:128], psv[0:3, 64:128], mn2b[0:3, 1:2],
                                        None, AO.add)
                nc.sync.dma_start(out=rows_sb[idx:idx + 1, 0:64], in_=stage[1:2, 0:64])
                nc.sync.dma_start(out=rows_sb[idx:idx + 1, 64:128], in_=stage[2:3, 64:128])

                vr_sb = psml.tile([7, 1], f32, tag="vr")
                nc.scalar.copy(vr_sb[:], psv[:, 130:131])
                nc.tensor.matmul(woin_ps[:, idx:idx + 1], vpT_sb[:], vr_sb[:],
                                 start=True, stop=False)
                nc.tensor.matmul(woin_ps[:, idx:idx + 1], vf_sb[:], awf[:],
                                 start=False, stop=True)

        # ---- tail: transpose rows, combine, wo matmul ----
        ictx.close()
        psO = ctx.enter_context(tc.tile_pool(name="psO", bufs=1, space="PSUM"))
        trp = psC.tile([128, 16], f32, tag="misc")
        nc.tensor.transpose(trp[:], rows_sb[:], id16[:])
        tr_sb = const.tile([128, 16], f32)
        nc.scalar.copy(tr_sb[:], trp[:])
        woin_sb = const.tile([128, 16], f32)
        nc.vector.tensor_tensor(woin_sb[:], tr_sb[:], woin_ps[:], AO.add)

        wo_sb = const.tile([128, HPC, HID], f32)
        nc.sync.dma_start(out=wo_sb[:], in_=woT[:].rearrange("(c p) n -> p c n", p=128))
        for half in range(2):
            po = psO.tile([B, HID // 2], f32, tag="po")
            for h in range(HPC):
                for nb in range(4):
                    j0 = half * 2048 + nb * 512
                    nc.tensor.matmul(po[:, nb * 512:(nb + 1) * 512],
                                     woin_sb[:, h * B:(h + 1) * B], wo_sb[:, h, j0:j0 + 512],
                                     start=(h == 0), stop=(h == HPC - 1))
            osb = const.tile([B, HID // 2], f32, tag=f"osb{half}")
            nc.scalar.copy(osb[:], po[:])
            nc.sync.dma_start(out=out[:, half * 2048:(half + 1) * 2048], in_=osb[:])

    nc.compile()
    return nc


def _host_prep(inputs):
    hs = np.asarray(inputs["hidden_states"], np.float32)
    pos = np.asarray(inputs["position_ids"])
    inv = 1.0 / (THETA ** (np.arange(0, D, 2, dtype=np.float32) / D))
    fr = pos[:, 0].astype(np.float32)[:, None] * inv[None, :]
    emb = np.concatenate([fr, fr], axis=1)
    cos_b = np.cos(emb).astype(np.float32)
    sin_b = np.sin(emb).astype(np.float32)
    cost = np.ascontiguousarray(np.tile(cos_b, (1, HPC)))
    sint = np.ascontiguousarray(np.tile(sin_b, (1, HPC)))
    hidT = np.ascontiguousarray(hs[:, 0, :].T)

    wq, wk, wv, wo = (np.asarray(inputs[k], np.float32) for k in ("wq", "wk", "wv", "wo"))
    in_maps = []
    for core in range(NCORES):
        h0 = core * HPC
        sl = slice(h0 * D, (h0 + HPC) * D)
        hsl = slice(h0, h0 + HPC)

        def rearr(x):  # [B,HPC,SQ,w] -> [B,HPC,128,NCH,w]
            w = x.shape[-1]
            return np.ascontiguousarray(
                x.reshape(B, HPC, NCH, 128, w).transpose(0, 1, 3, 2, 4))

        vp = np.asarray(inputs["value_p"], np.float32)[:, hsl]  # [B,HPC,D,R]
        vpT = np.zeros((B, HPC, 7, D), np.float32)
        vpT[:, :, 3:7, :] = vp.transpose(0, 1, 3, 2)
        m = {
            "hidT": hidT, "cost": cost, "sint": sint,
            "wqT": np.ascontiguousarray(wq[sl].T),
            "wkT": np.ascontiguousarray(wk[sl].T),
            "wvT": np.ascontiguousarray(wv[sl].T),
            "woT": np.ascontiguousarray(wo[:, sl].T),
            "kcode": np.ascontiguousarray(np.asarray(inputs["k_quant"], np.int32)[:, hsl]),
            "kscale": np.ascontiguousarray(np.asarray(inputs["k_scale"], np.float32)[:, hsl]),
            "kmn": np.ascontiguousarray(np.asarray(inputs["k_mn"], np.float32)[:, hsl]),
            "kfT": np.ascontiguousarray(
                np.asarray(inputs["k_full"], np.float32)[:, hsl].transpose(0, 1, 3, 2)),
            "kp": rearr(np.asarray(inputs["key_p"], np.float32)[:, hsl]),
            "keyq": np.ascontiguousarray(np.asarray(inputs["key_q"], np.float32)[:, hsl]),
            "vcode": np.ascontiguousarray(np.asarray(inputs["v_quant"], np.int32)[:, hsl]),
            "vscT": rearr(np.asarray(inputs["v_scale"], np.float32)[:, hsl]),
            "vmnT": rearr(np.asarray(inputs["v_mn"], np.float32)[:, hsl]),
            "vqT": rearr(np.asarray(inputs["value_q"], np.float32)[:, hsl]),
            "vpT": vpT,
            "vfr": np.ascontiguousarray(np.asarray(inputs["v_full"], np.float32)[:, hsl]),
        }
        in_maps.append(m)
    return in_maps


def kernel(**inputs):
    if "nc" not in _CACHE:
        _CACHE["nc"] = _build()
    nc = _CACHE["nc"]
    in_maps = _host_prep(inputs)
    res = run_bass_kernel_spmd(nc, in_maps, list(range(NCORES)),
                               trace=bool(os.environ.get("K_TRACE")))
    kernel.last = res
    total = np.zeros((B, HID), np.float32)
    for r in res.results:
        total += r["out"]
    return total.reshape(B, QL, HID)



# revision 14
# speedup vs baseline: 2.3398x; 2.3398x over previous
"""GEAR quantized-KV Llama attention decode step on 8 trn2 NeuronCores.

Sharding: tensor-parallel over heads (4 heads/core x 8 cores), all batches on
every core; each core computes a partial wo-product, summed on host.

v2: fp8 codes (exact for 0..15), bf16 weights, packed per-core blobs so every
DMA is large and contiguous; K-score matmuls use fp8 FWL stationary codes;
V matmuls keep head-dim on partitions; broadcasts via ones-matmul.
"""
import os
import sys
import math

sys.path.insert(0, "/opt/trn_rl_repo")
import numpy as np
import ml_dtypes
from contextlib import ExitStack

import concourse.bass as bass
import concourse.mybir as mybir
import concourse.tile as tile
from concourse import bacc, bass_isa
from concourse.bass_utils import run_bass_kernel_spmd
from concourse.masks import make_identity

B, H, D, HID = 4, 32, 128, 4096
SQ, SF, QL = 4096, 63, 1
GS, RANK = 64, 4
THETA = 10000.0
NCORES = 8
HPC = H // NCORES          # heads per core = 4
NP = B * HPC               # (b,h) pairs per core = 16
NCH = SQ // 128            # 32 s-chunks
G = SQ // GS               # 64 groups along seq (K side)
FD = D // GS               # 2 groups along head_dim (V side)
SFP = SF + 1               # 64 full-precision keys incl the new token
DT = mybir.dt
ISQD = 1.0 / math.sqrt(D)
KBW = G + G + 64 + NCH * RANK + RANK   # kblob width = 64+64+64+128+4 = 324
VBW = NCH * FD + 6 * NCH               # vblob width = 64+192 = 256

BF16 = ml_dtypes.bfloat16
FP8 = ml_dtypes.float8_e4m3

_CACHE = {}


def _build():
    nc = bacc.Bacc("TRN2", target_bir_lowering=False)
    f32, bf16, fp8 = DT.float32, DT.bfloat16, DT.float8e4

    hidq = nc.declare_dram_parameter("hidq", [128, NCH * B], bf16, isOutput=False)
    cosin = nc.declare_dram_parameter("cosin", [B, 2 * HPC * D], f32, isOutput=False)
    wqkv = nc.declare_dram_parameter("wqkv", [128, NCH * 3 * HPC * D], bf16, isOutput=False)
    woc = nc.declare_dram_parameter("woc", [128, HPC * HID], bf16, isOutput=False)
    kcode = nc.declare_dram_parameter("kcode", [NP, 128, SQ], fp8, isOutput=False)
    vcode = nc.declare_dram_parameter("vcode", [NP, 128, NCH * 128], fp8, isOutput=False)
    kblob = nc.declare_dram_parameter("kblob", [128, NP * KBW], bf16, isOutput=False)
    vblob = nc.declare_dram_parameter("vblob", [128, NP * VBW], bf16, isOutput=False)
    vpm = nc.declare_dram_parameter("vpm", [6, NP * 128], bf16, isOutput=False)
    vfull = nc.declare_dram_parameter("vfull", [SF, NP * 128], bf16, isOutput=False)
    out = nc.declare_dram_parameter("out", [B, HID], f32, isOutput=True)

    AO = mybir.AluOpType
    AF = mybir.ActivationFunctionType

    with tile.TileContext(nc) as tc, ExitStack() as ctx:
        const = ctx.enter_context(tc.tile_pool(name="const", bufs=1))
        pw = ctx.enter_context(tc.tile_pool(name="pw", bufs=2))
        ictx = ctx.enter_context(ExitStack())
        psml = ictx.enter_context(tc.tile_pool(name="psml", bufs=3))
        pkc = ictx.enter_context(tc.tile_pool(name="pkc", bufs=2))
        pvt = ictx.enter_context(tc.tile_pool(name="pvt", bufs=2))
        psA = ictx.enter_context(tc.tile_pool(name="psA", bufs=2, space="PSUM"))
        psB = ictx.enter_context(tc.tile_pool(name="psB", bufs=2, space="PSUM"))
        pctx = ExitStack()
        psC = pctx.enter_context(tc.tile_pool(name="psC", bufs=1, space="PSUM"))

        # ---- constants / resident blobs ----
        id4 = const.tile([4, 4], f32)
        make_identity(nc, id4[:])
        ones_row = const.tile([1, 128], bf16)
        nc.vector.memset(ones_row[:], 1.0)
        ones_col = const.tile([128, 1], f32)
        nc.vector.memset(ones_col[:], 1.0)

        hid_sb = const.tile([128, NCH, B], bf16)
        nc.sync.dma_start(out=hid_sb[:], in_=hidq[:].rearrange("p (c b) -> p c b", b=B))
        cos_sb = const.tile([B, HPC * D], f32)
        nc.sync.dma_start(out=cos_sb[:], in_=cosin[:, 0:HPC * D])
        sin_sb = const.tile([B, HPC * D], f32)
        nc.sync.dma_start(out=sin_sb[:], in_=cosin[:, HPC * D:2 * HPC * D])
        kblob_sb = const.tile([128, NP, KBW], bf16)
        nc.sync.dma_start(out=kblob_sb[:], in_=kblob[:].rearrange("p (n w) -> p n w", w=KBW))
        vblob_sb = const.tile([128, NP, VBW], bf16)
        nc.sync.dma_start(out=vblob_sb[:], in_=vblob[:].rearrange("p (n w) -> p n w", w=VBW))
        vpm_sb = const.tile([6, NP, 128], bf16)
        nc.sync.dma_start(out=vpm_sb[:], in_=vpm[:].rearrange("p (n d) -> p n d", d=128))
        vfull_sb = const.tile([SFP, NP, 128], bf16)
        nc.sync.dma_start(out=vfull_sb[0:SF], in_=vfull[:].rearrange("p (n d) -> p n d", d=128))
        # wo streams on the ACT ring so it doesn't block pair-code DMAs
        wo_sb = const.tile([128, HPC, HID], bf16)
        nc.scalar.dma_start(out=wo_sb[:], in_=woc[:].rearrange("p (h n) -> p h n", n=HID))

        # ---- projections: psum[b, 1536] = sum_c hid_c^T @ wqkv_c ----
        pps = psC.tile([B, 3 * HPC * D], f32, tag="proj")
        for blk in range(4):
            slab = pw.tile([128, 8, 3 * HPC * D], bf16, tag="wslab")
            nc.scalar.dma_start(
                out=slab[:],
                in_=wqkv[:, 8 * blk * 1536:(8 * blk + 8) * 1536].rearrange(
                    "p (c n) -> p c n", n=1536),
            )
            for j in range(8):
                c = 8 * blk + j
                for nb in range(3):
                    nc.tensor.matmul(pps[:, nb * 512:(nb + 1) * 512],
                                     hid_sb[:, c, :], slab[:, j, nb * 512:(nb + 1) * 512],
                                     start=(c == 0), stop=(c == 31))
        qkv_sb = const.tile([B, 3 * HPC * D], f32)
        nc.scalar.copy(qkv_sb[:], pps[:])
        q_sb = qkv_sb[:, 0:512]
        k_sb = qkv_sb[:, 512:1024]

        # ---- RoPE on q and k (rows [B, HPC*D], f32) ----
        def rope(x_v, tagp):
            rot = const.tile([B, HPC * D], f32, tag=tagp + "rot")
            xv = x_v.rearrange("b (h two d) -> b h two d", two=2, d=64)
            rv = rot[:].rearrange("b (h two d) -> b h two d", two=2, d=64)
            nc.vector.tensor_scalar(rv[:, :, 0, :], xv[:, :, 1, :], -1.0, None, AO.mult)
            nc.vector.tensor_copy(rv[:, :, 1, :], xv[:, :, 0, :])
            nc.vector.tensor_tensor(rot[:], rot[:], sin_sb[:], AO.mult)
            ro = const.tile([B, HPC * D], f32, tag=tagp + "ro")
            nc.vector.tensor_tensor(ro[:], x_v, cos_sb[:], AO.mult)
            nc.vector.tensor_tensor(ro[:], ro[:], rot[:], AO.add)
            return ro
        qro = rope(q_sb, "q")
        kro = rope(k_sb, "k")

        # per-head transposed columns: qscT [128, h, b] (scaled by 1/sqrt(D)), kT
        qscT = const.tile([128, HPC, B], bf16)
        qsc32 = const.tile([128, HPC, B], f32)
        kT = const.tile([128, HPC, B], bf16)
        for h in range(HPC):
            pq = psC.tile([128, B], f32, tag="tr")
            nc.tensor.transpose(pq[:], qro[0:B, h * D:(h + 1) * D], id4[:])
            nc.scalar.mul(qscT[:, h, :], pq[:], ISQD)
            nc.scalar.mul(qsc32[:, h, :], pq[:], ISQD)
            pk = psC.tile([128, B], f32, tag="tr")
            nc.tensor.transpose(pk[:], kro[0:B, h * D:(h + 1) * D], id4[:])
            nc.scalar.copy(kT[:, h, :], pk[:])
        pctx.close()

        # new-token k/v into the blobs (k: DVE copy; v: tiny cast DMA per pair)
        for b in range(B):
            for h in range(HPC):
                p = b * HPC + h
                nc.vector.tensor_copy(kblob_sb[:, p, 191:192], kT[:, h, b:b + 1])
                nc.gpsimd.dma_start(
                    out=vfull_sb[SF:SFP, p, :],
                    in_=qkv_sb[b:b + 1, 1024 + h * D:1024 + (h + 1) * D])

        woin = const.tile([128, NP], bf16)

        # ---- per (b, h) attention ----
        for b in range(B):
            for h in range(HPC):
                p = b * HPC + h
                idx = h * B + b
                qcol = qscT[:, h, b:b + 1]

                kc = pkc.tile([128, SQ], fp8, tag="kc")
                nc.sync.dma_start(out=kc[:], in_=kcode[p])
                vt = pvt.tile([128, NCH, 128], fp8, tag="vt")
                nc.sync.dma_start(out=vt[:], in_=vcode[p])

                ksc = kblob_sb[:, p, 0:G]
                kmn_v = kblob_sb[:, p, G:2 * G]
                kfp = kblob_sb[:, p, 128:128 + SFP]
                kp_v = kblob_sb[:, p, 192:192 + NCH * RANK].rearrange(
                    "p (c r) -> p c r", r=RANK)
                keyq_v = kblob_sb[:, p, 320:324]
                vsc = vblob_sb[:, p, 0:NCH * FD].rearrange("p (c g) -> p c g", g=FD)
                vqmn = vblob_sb[:, p, NCH * FD:VBW].rearrange("p (j c) -> p j c", c=NCH)

                # quant K scores: psk[s, 2c + g'] per chunk
                qs = psml.tile([128, G], bf16, tag="qs")
                nc.vector.tensor_scalar(qs[:], ksc, qsc32[:, h, b:b + 1], None, AO.mult)
                pska = psA.tile([128, 66], f32, tag="psk")
                psk = pska[:, 0:64]
                psv = pska[:, 64:66]
                for c in range(NCH):
                    nc.tensor.matmul(psk[:, 2 * c:2 * c + 2], kc[:, c * 128:(c + 1) * 128],
                                     qs[:, 2 * c:2 * c + 2], start=True, stop=True)

                misc = psB.tile([128, 139], f32, tag="misc")
                # kf scores [64, 1]
                nc.tensor.matmul(misc[0:SFP, 0:1], kfp, qcol, start=True, stop=True)
                # qr row [1, 1:5], mn bias row [1, 5:69]
                nc.tensor.matmul(misc[0:1, 1:5], qcol, keyq_v, start=True, stop=True)
                nc.tensor.matmul(misc[0:1, 5:69], qcol, kmn_v, start=True, stop=True)
                qb_row = psml.tile([1, 68], bf16, tag="qbrow")
                nc.scalar.copy(qb_row[:], misc[0:1, 1:69])
                # broadcast to all partitions: [128, 69:73] = qr, [128, 73:137] = bias
                psb = misc[:, 69:137]
                nc.tensor.matmul(psb, ones_row[:], qb_row[:], start=True, stop=True)

                # low-rank correction lr[s, c] = sum_r kp[s,c,r] * qr[r]
                lrt = psml.tile([128, NCH, RANK], f32, tag="lrt")
                nc.vector.tensor_tensor(lrt[:], kp_v,
                                        misc[:, None, 69:73].to_broadcast((128, NCH, RANK)),
                                        AO.mult)
                lr = psml.tile([128, NCH], f32, tag="lr")
                nc.vector.reduce_sum(lr[:], lrt[:], axis=mybir.AxisListType.X)

                att = psml.tile([128, NCH + 1], f32, tag="att")
                pskv = psk.rearrange("p (c two) -> p c two", two=2)
                bbv = misc[:, 73:137].rearrange("p (c two) -> p c two", two=2)
                nc.vector.tensor_tensor(att[0:64, 0:NCH], pskv[0:64, :, 0], lr[0:64, :], AO.add)
                nc.vector.tensor_tensor(att[0:64, 0:NCH], att[0:64, 0:NCH], bbv[0:64, :, 0], AO.add)
                nc.vector.tensor_tensor(att[64:128, 0:NCH], pskv[64:128, :, 1], lr[64:128, :], AO.add)
                nc.vector.tensor_tensor(att[64:128, 0:NCH], att[64:128, 0:NCH], bbv[64:128, :, 1], AO.add)
                nc.vector.memset(att[:, NCH:NCH + 1], -1e9)
                nc.vector.tensor_copy(att[0:SFP, NCH:NCH + 1], misc[0:SFP, 0:1])

                # softmax over all 128 x 33 entries
                m1 = psml.tile([128, 1], f32, tag="m1")
                nc.vector.reduce_max(m1[:], att[:], axis=mybir.AxisListType.X)
                mg = psml.tile([128, 1], f32, tag="mg")
                nc.gpsimd.partition_all_reduce(mg[:], m1[:], 128, bass_isa.ReduceOp.max)
                negm = psml.tile([128, 1], f32, tag="negm")
                nc.vector.tensor_scalar(negm[:], mg[:], -1.0, None, AO.mult)
                e = psml.tile([128, NCH + 1], bf16, tag="e")
                ssum = psml.tile([128, 1], f32, tag="ssum")
                nc.scalar.activation(e[:], att[:], AF.Exp, bias=negm[:, 0:1], scale=1.0,
                                     alpha=0.0, accum_out=ssum[:])
                sg = psml.tile([128, 1], f32, tag="sg")
                nc.gpsimd.partition_all_reduce(sg[:], ssum[:], 128, bass_isa.ReduceOp.add)
                recip = psml.tile([128, 1], f32, tag="recip")
                nc.vector.reciprocal(recip[:], sg[:])

                # moving cols for V matmuls: aw * vscale per chunk
                ev = e[:, 0:NCH, None]
                awvs = psml.tile([128, NCH, FD], bf16, tag="awvs")
                nc.vector.scalar_tensor_tensor(awvs[:], ev.to_broadcast((128, NCH, FD)),
                                               recip[:, 0:1], vsc, AO.mult, AO.mult)
                awn = psml.tile([128, NCH], bf16, tag="awn")
                nc.vector.tensor_scalar(awn[:], e[:, 0:NCH], recip[:, 0:1], None, AO.mult)
                awf2 = psml.tile([SFP, FD], bf16, tag="awf")
                nc.vector.tensor_scalar(awf2[:],
                                        e[0:SFP, NCH:NCH + 1].to_broadcast((SFP, FD)),
                                        recip[0:SFP, 0:1], None, AO.mult)

                # vq/vmn contractions: per-partition partials then ones-matmul
                prod6 = psml.tile([128, 6, NCH], f32, tag="prod6")
                nc.vector.tensor_tensor(prod6[:], vqmn,
                                        awn[:, None, :].to_broadcast((128, 6, NCH)), AO.mult)
                part6 = psml.tile([128, 6], f32, tag="part6")
                nc.vector.reduce_sum(part6[:], prod6[:], axis=mybir.AxisListType.X)
                nc.tensor.matmul(misc[0:6, 137:138], part6[:], ones_col[:],
                                 start=True, stop=True)
                rvec2 = psml.tile([6, FD], bf16, tag="rvec")
                nc.vector.tensor_copy(rvec2[:], misc[0:6, 137:138].to_broadcast((6, FD)))

                # V matmuls: quant chunks + residual + low-rank + mn corrections,
                # all accumulated into psv[d, 0:2] (corrections apply to both cols)
                for c in range(NCH):
                    nc.tensor.matmul(psv, vt[:, c, :], awvs[:, c, :],
                                     start=(c == 0), stop=False)
                nc.tensor.matmul(psv, vfull_sb[:, p, :], awf2[:], start=False, stop=False)
                nc.tensor.matmul(psv, vpm_sb[:, p, :], rvec2[:], start=False, stop=True)

                nc.vector.tensor_copy(woin[0:64, idx:idx + 1], psv[0:64, 0:1])
                nc.vector.tensor_copy(woin[64:128, idx:idx + 1], psv[64:128, 1:2])

        # ---- tail: wo matmul ----
        ictx.close()
        psO = ctx.enter_context(tc.tile_pool(name="psO", bufs=2, space="PSUM"))
        for half in range(2):
            po = psO.tile([B, HID // 2], f32, tag="po")
            for h in range(HPC):
                for nb in range(4):
                    j0 = half * 2048 + nb * 512
                    nc.tensor.matmul(po[:, nb * 512:(nb + 1) * 512],
                                     woin[:, h * B:(h + 1) * B], wo_sb[:, h, j0:j0 + 512],
                                     start=(h == 0), stop=(h == HPC - 1))
            osb = const.tile([B, HID // 2], f32, tag=f"osb{half}")
            nc.scalar.copy(osb[:], po[:])
            nc.sync.dma_start(out=out[:, half * 2048:(half + 1) * 2048], in_=osb[:])

    nc.compile()
    return nc


def _host_prep(inputs):
    hs = np.asarray(inputs["hidden_states"], np.float32)
    pos = np.asarray(inputs["position_ids"])
    inv = 1.0 / (THETA ** (np.arange(0, D, 2, dtype=np.float32) / D))
    fr = pos[:, 0].astype(np.float32)[:, None] * inv[None, :]
    emb = np.concatenate([fr, fr], axis=1)
    cos_b = np.cos(emb).astype(np.float32)
    sin_b = np.sin(emb).astype(np.float32)
    cosin = np.ascontiguousarray(
        np.concatenate([np.tile(cos_b, (1, HPC)), np.tile(sin_b, (1, HPC))], axis=1))
    # hidq [128, NCH*B] bf16
    hidq = np.ascontiguousarray(
        hs[:, 0, :].T.reshape(NCH, 128, B).transpose(1, 0, 2).reshape(128, NCH * B)
    ).astype(BF16)

    wq, wk, wv, wo = (np.asarray(inputs[k], np.float32) for k in ("wq", "wk", "wv", "wo"))
    kq_f = np.asarray(inputs["k_quant"])
    vq_f = np.asarray(inputs["v_quant"])
    ks_f = np.asarray(inputs["k_scale"], np.float32)
    km_f = np.asarray(inputs["k_mn"], np.float32)
    kf_f = np.asarray(inputs["k_full"], np.float32)
    kp_f = np.asarray(inputs["key_p"], np.float32)
    kqr_f = np.asarray(inputs["key_q"], np.float32)
    vs_f = np.asarray(inputs["v_scale"], np.float32)
    vm_f = np.asarray(inputs["v_mn"], np.float32)
    vf_f = np.asarray(inputs["v_full"], np.float32)
    vqv_f = np.asarray(inputs["value_q"], np.float32)
    vp_f = np.asarray(inputs["value_p"], np.float32)

    in_maps = []
    for core in range(NCORES):
        h0 = core * HPC
        sl = slice(h0 * D, (h0 + HPC) * D)
        hsl = slice(h0, h0 + HPC)

        wqkv = np.concatenate([wq[sl].T, wk[sl].T, wv[sl].T], axis=1)  # [4096, 1536]
        wqkv = wqkv.reshape(NCH, 128, 3 * HPC * D).transpose(1, 0, 2).reshape(128, -1)
        woc = wo[:, sl].T.reshape(HPC, 128, HID).transpose(1, 0, 2).reshape(128, -1)

        kcode = kq_f[:, hsl].reshape(NP, 128, SQ).astype(FP8)
        # vcode: [B,HPC,SQ,D] -> [pair, p=s%128, c, d]
        vcode = (vq_f[:, hsl].reshape(B, HPC, NCH, 128, D)
                 .transpose(0, 1, 3, 2, 4).reshape(NP, 128, NCH * 128).astype(FP8))

        kblob = np.zeros((128, NP, KBW), np.float32)
        kblob[:, :, 0:G] = ks_f[:, hsl].reshape(NP, 128, G).transpose(1, 0, 2)
        kblob[:, :, G:2 * G] = km_f[:, hsl].reshape(NP, 128, G).transpose(1, 0, 2)
        kblob[:, :, 128:128 + SF] = kf_f[:, hsl].reshape(NP, SF, 128).transpose(2, 0, 1)
        kblob[:, :, 192:192 + NCH * RANK] = (
            kp_f[:, hsl].reshape(B, HPC, NCH, 128, RANK)
            .transpose(3, 0, 1, 2, 4).reshape(128, NP, NCH * RANK))
        kblob[:, :, 320:324] = kqr_f[:, hsl].reshape(NP, 128, RANK).transpose(1, 0, 2)

        vblob = np.zeros((128, NP, VBW), np.float32)
        vblob[:, :, 0:NCH * FD] = (
            vs_f[:, hsl].reshape(B, HPC, NCH, 128, FD)
            .transpose(3, 0, 1, 2, 4).reshape(128, NP, NCH * FD))
        vblob[:, :, NCH * FD:NCH * FD + 4 * NCH] = (
            vqv_f[:, hsl].reshape(B, HPC, NCH, 128, RANK)
            .transpose(3, 0, 1, 4, 2).reshape(128, NP, 4 * NCH))
        vblob[:, :, NCH * FD + 4 * NCH:VBW] = (
            vm_f[:, hsl].reshape(B, HPC, NCH, 128, FD)
            .transpose(3, 0, 1, 4, 2).reshape(128, NP, 2 * NCH))

        vpm = np.zeros((6, NP, 128), np.float32)
        vpm[0:4] = vp_f[:, hsl].reshape(NP, 128, RANK).transpose(2, 0, 1)
        vpm[4, :, 0:64] = 1.0
        vpm[5, :, 64:128] = 1.0

        vfull = vf_f[:, hsl].reshape(NP, SF, 128).transpose(1, 0, 2)

        m = {
            "hidq": hidq, "cosin": cosin,
            "wqkv": np.ascontiguousarray(wqkv).astype(BF16),
            "woc": np.ascontiguousarray(woc).astype(BF16),
            "kcode": np.ascontiguousarray(kcode),
            "vcode": np.ascontiguousarray(vcode),
            "kblob": kblob.reshape(128, NP * KBW).astype(BF16),
            "vblob": vblob.reshape(128, NP * VBW).astype(BF16),
            "vpm": vpm.reshape(6, NP * 128).astype(BF16),
            "vfull": np.ascontiguousarray(vfull).reshape(SF, NP * 128).astype(BF16),
        }
        in_maps.append(m)
    return in_maps


def kernel(**inputs):
    if "nc" not in _CACHE:
        _CACHE["nc"] = _build()
    nc = _CACHE["nc"]
    in_maps = _host_prep(inputs)
    res = run_bass_kernel_spmd(nc, in_maps, list(range(NCORES)),
                               trace=bool(os.environ.get("K_TRACE")))
    kernel.last = res
    total = np.zeros((B, HID), np.float32)
    for r in res.results:
        total += r["out"]
    return total.reshape(B, QL, HID)


# revision 26
# speedup vs baseline: 2.5003x; 1.0686x over previous
"""GEAR quantized-KV Llama attention decode step on 8 trn2 NeuronCores.

Sharding: tensor-parallel over heads (4 heads/core x 8 cores), all batches on
every core; each core computes a partial wo-product, summed on host.

v2: fp8 codes (exact for 0..15), bf16 weights, packed per-core blobs so every
DMA is large and contiguous; K-score matmuls use fp8 FWL stationary codes;
V matmuls keep head-dim on partitions; broadcasts via ones-matmul.
"""
import os
import sys
import math

sys.path.insert(0, "/opt/trn_rl_repo")
import numpy as np
import ml_dtypes
from contextlib import ExitStack

import concourse.bass as bass
import concourse.mybir as mybir
import concourse.tile as tile
from concourse import bacc, bass_isa
from concourse.bass_utils import run_bass_kernel_spmd
from concourse.masks import make_identity

B, H, D, HID = 4, 32, 128, 4096
SQ, SF, QL = 4096, 63, 1
GS, RANK = 64, 4
THETA = 10000.0
NCORES = 8
HPC = H // NCORES          # heads per core = 4
NP = B * HPC               # (b,h) pairs per core = 16
NCH = SQ // 128            # 32 s-chunks
G = SQ // GS               # 64 groups along seq (K side)
FD = D // GS               # 2 groups along head_dim (V side)
SFP = SF + 1               # 64 full-precision keys incl the new token
DT = mybir.dt
ISQD = 1.0 / math.sqrt(D)
KBW = G + G + 64 + NCH * RANK + RANK   # kblob width = 64+64+64+128+4 = 324
VBW = NCH * FD + 6 * NCH               # vblob width = 64+192 = 256

BF16 = ml_dtypes.bfloat16
FP8 = ml_dtypes.float8_e4m3

_CACHE = {}


def _build():
    nc = bacc.Bacc("TRN2", target_bir_lowering=False)
    f32, bf16, fp8 = DT.float32, DT.bfloat16, DT.float8e4

    hidq = nc.declare_dram_parameter("hidq", [128, NCH * B], bf16, isOutput=False)
    cosin = nc.declare_dram_parameter("cosin", [B, 2 * HPC * D], f32, isOutput=False)
    wqkv = nc.declare_dram_parameter("wqkv", [128, NCH * 3 * HPC * D], bf16, isOutput=False)
    woc = nc.declare_dram_parameter("woc", [128, HPC * HID], bf16, isOutput=False)
    codes = nc.declare_dram_parameter("codes", [NP, 128, 2 * SQ], fp8, isOutput=False)
    kblob = nc.declare_dram_parameter("kblob", [128, NP * KBW], bf16, isOutput=False)
    vblob = nc.declare_dram_parameter("vblob", [128, NP * VBW], bf16, isOutput=False)
    vpm = nc.declare_dram_parameter("vpm", [6, NP * 128], bf16, isOutput=False)
    vfull = nc.declare_dram_parameter("vfull", [SF, NP * 128], bf16, isOutput=False)
    out = nc.declare_dram_parameter("out", [B, HID], f32, isOutput=True)

    AO = mybir.AluOpType
    AF = mybir.ActivationFunctionType

    with tile.TileContext(nc) as tc, ExitStack() as ctx:
        const = ctx.enter_context(tc.tile_pool(name="const", bufs=1))
        pw = ctx.enter_context(tc.tile_pool(name="pw", bufs=2))
        ictx = ctx.enter_context(ExitStack())
        psml = ictx.enter_context(tc.tile_pool(name="psml", bufs=4))
        pkc = ictx.enter_context(tc.tile_pool(name="pkc", bufs=3))
        psA = ictx.enter_context(tc.tile_pool(name="psA", bufs=4, space="PSUM"))
        pctx = ExitStack()
        psC = pctx.enter_context(tc.tile_pool(name="psC", bufs=1, space="PSUM"))

        # ---- constants / resident blobs ----
        id4 = const.tile([4, 4], f32)
        make_identity(nc, id4[:])
        ones_row = const.tile([1, 128], bf16)
        nc.vector.memset(ones_row[:], 1.0)
        ones_col = const.tile([128, 1], f32)
        nc.vector.memset(ones_col[:], 1.0)

        hid_sb = const.tile([128, NCH, B], bf16)
        nc.sync.dma_start(out=hid_sb[:], in_=hidq[:].rearrange("p (c b) -> p c b", b=B))
        cos_sb = const.tile([B, HPC * D], f32)
        nc.sync.dma_start(out=cos_sb[:], in_=cosin[:, 0:HPC * D])
        sin_sb = const.tile([B, HPC * D], f32)
        nc.sync.dma_start(out=sin_sb[:], in_=cosin[:, HPC * D:2 * HPC * D])
        kblob_sb = const.tile([128, NP, KBW], bf16)
        nc.sync.dma_start(out=kblob_sb[:], in_=kblob[:].rearrange("p (n w) -> p n w", w=KBW))
        vblob_sb = const.tile([128, NP, VBW], bf16)
        nc.sync.dma_start(out=vblob_sb[:], in_=vblob[:].rearrange("p (n w) -> p n w", w=VBW))
        vpm_sb = const.tile([6, NP, 128], bf16)
        nc.sync.dma_start(out=vpm_sb[:], in_=vpm[:].rearrange("p (n d) -> p n d", d=128))
        vfull_sb = const.tile([SFP, NP, 128], bf16)
        nc.sync.dma_start(out=vfull_sb[0:SF], in_=vfull[:].rearrange("p (n d) -> p n d", d=128))
        wo_sb = const.tile([128, HPC, HID], bf16)

        # ---- projections: psum[b, 1536] = sum_c hid_c^T @ wqkv_c ----
        pps = psC.tile([B, 3 * HPC * D], f32, tag="proj")
        for blk in range(4):
            slab = pw.tile([128, 8, 3 * HPC * D], bf16, tag="wslab")
            nc.scalar.dma_start(
                out=slab[:],
                in_=wqkv[:, 8 * blk * 1536:(8 * blk + 8) * 1536].rearrange(
                    "p (c n) -> p c n", n=1536),
            )
            for j in range(8):
                c = 8 * blk + j
                for nb in range(3):
                    nc.tensor.matmul(pps[:, nb * 512:(nb + 1) * 512],
                                     hid_sb[:, c, :], slab[:, j, nb * 512:(nb + 1) * 512],
                                     start=(c == 0), stop=(c == 31))
        qkv_sb = const.tile([B, 3 * HPC * D], f32)
        nc.scalar.copy(qkv_sb[:], pps[:])
        q_sb = qkv_sb[:, 0:512]
        k_sb = qkv_sb[:, 512:1024]

        # ---- RoPE on q and k (rows [B, HPC*D], f32) ----
        def rope(x_v, tagp):
            rot = const.tile([B, HPC * D], f32, tag=tagp + "rot")
            xv = x_v.rearrange("b (h two d) -> b h two d", two=2, d=64)
            rv = rot[:].rearrange("b (h two d) -> b h two d", two=2, d=64)
            nc.vector.tensor_scalar(rv[:, :, 0, :], xv[:, :, 1, :], -1.0, None, AO.mult)
            nc.vector.tensor_copy(rv[:, :, 1, :], xv[:, :, 0, :])
            nc.vector.tensor_tensor(rot[:], rot[:], sin_sb[:], AO.mult)
            ro = const.tile([B, HPC * D], f32, tag=tagp + "ro")
            nc.vector.tensor_tensor(ro[:], x_v, cos_sb[:], AO.mult)
            nc.vector.tensor_tensor(ro[:], ro[:], rot[:], AO.add)
            return ro
        qro = rope(q_sb, "q")
        kro = rope(k_sb, "k")

        # per-head transposed columns: qscT [128, h, b] (scaled by 1/sqrt(D)), kT
        qscT = const.tile([128, HPC, B], bf16)
        qsc32 = const.tile([128, HPC, B], f32)
        kT = const.tile([128, HPC, B], bf16)
        for h in range(HPC):
            pq = psC.tile([128, B], f32, tag="tr")
            nc.tensor.transpose(pq[:], qro[0:B, h * D:(h + 1) * D], id4[:])
            nc.scalar.mul(qscT[:, h, :], pq[:], ISQD)
            nc.scalar.mul(qsc32[:, h, :], pq[:], ISQD)
            pk = psC.tile([128, B], f32, tag="tr")
            nc.tensor.transpose(pk[:], kro[0:B, h * D:(h + 1) * D], id4[:])
            nc.scalar.copy(kT[:, h, :], pk[:])
        pctx.close()

        # new-token k/v into the blobs (k: DVE copy; v: tiny cast DMA per pair)
        for b in range(B):
            for h in range(HPC):
                p = b * HPC + h
                nc.vector.tensor_copy(kblob_sb[:, p, 195:196], kT[:, h, b:b + 1])
                nc.gpsimd.dma_start(
                    out=vfull_sb[SF:SFP, p, :],
                    in_=qkv_sb[b:b + 1, 1024 + h * D:1024 + (h + 1) * D])

        woin = const.tile([128, NP], bf16)

        # ---- per (b, h) attention ----
        for b in range(B):
            for h in range(HPC):
                p = b * HPC + h
                idx = h * B + b
                qcol = qscT[:, h, b:b + 1]
                if p == 4:
                    # wo weights stream during attention, on the ACT ring
                    nc.scalar.dma_start(out=wo_sb[:],
                                        in_=woc[:].rearrange("p (h n) -> p h n", n=HID))

                cds = pkc.tile([128, 2, SQ], fp8, tag="codes")
                nc.sync.dma_start(out=cds[:], in_=codes[p].rearrange(
                    "p (two s) -> p two s", s=SQ))
                kc = cds[:, 0, :]
                vt = cds[:, 1, :].rearrange("p (c d) -> p c d", d=128)

                ksc = kblob_sb[:, p, 0:G]
                kmnq = kblob_sb[:, p, G:G + 68]            # kmn | keyq
                kfp = kblob_sb[:, p, 132:132 + SFP]
                kp_v = kblob_sb[:, p, 196:196 + NCH * RANK].rearrange(
                    "p (c r) -> p c r", r=RANK)
                vsc = vblob_sb[:, p, 0:NCH * FD].rearrange("p (c g) -> p c g", g=FD)
                vqmn = vblob_sb[:, p, NCH * FD:VBW].rearrange("p (j c) -> p j c", c=NCH)

                pp = psA.tile([128, 140], f32, tag="pp")
                psk = pp[:, 0:64]
                psv = pp[:, 64:66]

                # kf scores [64, 1]
                nc.tensor.matmul(pp[0:SFP, 66:67], kfp, qcol, start=True, stop=True)
                # mn bias row [1, 67:131], qr row [1, 131:135]
                nc.tensor.matmul(pp[0:1, 67:135], qcol, kmnq, start=True, stop=True)
                qb_row = psml.tile([1, 68], bf16, tag="qbrow")
                nc.scalar.copy(qb_row[:], pp[0:1, 67:135])
                # bias broadcast seeds psk; chunk matmuls accumulate onto it.
                # qr broadcast to [128, 135:139]
                nc.tensor.matmul(psk, ones_row[:], qb_row[0:1, 0:64],
                                 start=True, stop=False, skip_group_check=True)
                nc.tensor.matmul(pp[:, 135:139], ones_row[:], qb_row[0:1, 64:68],
                                 start=True, stop=True)

                # quant K scores: psk[s, 2c + g'] += codes^T qs per chunk
                qs = psml.tile([128, G], bf16, tag="qs")
                nc.vector.tensor_scalar(qs[:], ksc, qsc32[:, h, b:b + 1], None, AO.mult)
                for c in range(NCH):
                    nc.tensor.matmul(psk[:, 2 * c:2 * c + 2], kc[:, c * 128:(c + 1) * 128],
                                     qs[:, 2 * c:2 * c + 2], start=False,
                                     stop=(c == NCH - 1), skip_group_check=True)

                # low-rank correction lr[s, c] = sum_r kp[s,c,r] * qr[r]
                lrt = psml.tile([128, NCH, RANK], f32, tag="lrt")
                nc.vector.tensor_tensor(lrt[:], kp_v,
                                        pp[:, None, 135:139].to_broadcast((128, NCH, RANK)),
                                        AO.mult)
                lr = psml.tile([128, NCH], f32, tag="lr")
                nc.vector.reduce_sum(lr[:], lrt[:], axis=mybir.AxisListType.X)

                att = psml.tile([128, NCH + 1], f32, tag="att")
                pskv = psk.rearrange("p (c two) -> p c two", two=2)
                nc.vector.tensor_tensor(att[0:64, 0:NCH], pskv[0:64, :, 0], lr[0:64, :], AO.add)
                nc.vector.tensor_tensor(att[64:128, 0:NCH], pskv[64:128, :, 1], lr[64:128, :], AO.add)
                nc.vector.memset(att[:, NCH:NCH + 1], -1e9)
                nc.vector.tensor_copy(att[0:SFP, NCH:NCH + 1], pp[0:SFP, 66:67])

                # softmax over all 128 x 33 entries
                m1 = psml.tile([128, 1], f32, tag="m1")
                nc.vector.reduce_max(m1[:], att[:], axis=mybir.AxisListType.X)
                mg = psml.tile([128, 1], f32, tag="mg")
                nc.gpsimd.partition_all_reduce(mg[:], m1[:], 128, bass_isa.ReduceOp.max)
                negm = psml.tile([128, 1], f32, tag="negm")
                nc.vector.tensor_scalar(negm[:], mg[:], -1.0, None, AO.mult)
                e = psml.tile([128, NCH + 1], bf16, tag="e")
                ssum = psml.tile([128, 1], f32, tag="ssum")
                nc.scalar.activation(e[:], att[:], AF.Exp, bias=negm[:, 0:1], scale=1.0,
                                     alpha=0.0, accum_out=ssum[:])
                sg = psml.tile([128, 1], f32, tag="sg")
                nc.gpsimd.partition_all_reduce(sg[:], ssum[:], 128, bass_isa.ReduceOp.add)
                recip = psml.tile([128, 1], f32, tag="recip")
                nc.vector.reciprocal(recip[:], sg[:])

                # moving cols for V matmuls: aw * vscale per chunk
                ev = e[:, 0:NCH, None]
                awvs = psml.tile([128, NCH, FD], bf16, tag="awvs")
                nc.vector.scalar_tensor_tensor(awvs[:], ev.to_broadcast((128, NCH, FD)),
                                               recip[:, 0:1], vsc, AO.mult, AO.mult)
                awn = psml.tile([128, NCH], bf16, tag="awn")
                nc.vector.tensor_scalar(awn[:], e[:, 0:NCH], recip[:, 0:1], None, AO.mult)
                awf2 = psml.tile([SFP, FD], bf16, tag="awf")
                nc.vector.tensor_scalar(awf2[:],
                                        e[0:SFP, NCH:NCH + 1].to_broadcast((SFP, FD)),
                                        recip[0:SFP, 0:1], None, AO.mult)

                # vq/vmn contractions: per-partition partials then ones-matmul
                prod6 = psml.tile([128, 6, NCH], f32, tag="prod6")
                nc.vector.tensor_tensor(prod6[:], vqmn,
                                        awn[:, None, :].to_broadcast((128, 6, NCH)), AO.mult)
                part6 = psml.tile([128, 6], f32, tag="part6")
                nc.vector.reduce_sum(part6[:], prod6[:], axis=mybir.AxisListType.X)
                nc.tensor.matmul(pp[0:6, 139:140], part6[:], ones_col[:],
                                 start=True, stop=True)
                rvec2 = psml.tile([6, FD], bf16, tag="rvec")
                nc.vector.tensor_copy(rvec2[:], pp[0:6, 139:140].to_broadcast((6, FD)))

                # V matmuls: quant chunks + residual + low-rank + mn corrections,
                # all accumulated into psv[d, 0:2] (corrections apply to both cols)
                for c in range(NCH):
                    nc.tensor.matmul(psv, vt[:, c, :], awvs[:, c, :],
                                     start=(c == 0), stop=False)
                nc.tensor.matmul(psv, vfull_sb[:, p, :], awf2[:], start=False, stop=False)
                nc.tensor.matmul(psv, vpm_sb[:, p, :], rvec2[:], start=False, stop=True)

                nc.vector.tensor_copy(woin[0:64, idx:idx + 1], psv[0:64, 0:1])
                nc.vector.tensor_copy(woin[64:128, idx:idx + 1], psv[64:128, 1:2])

        # ---- tail: wo matmul ----
        ictx.close()
        psO = ctx.enter_context(tc.tile_pool(name="psO", bufs=2, space="PSUM"))
        for half in range(2):
            po = psO.tile([B, HID // 2], f32, tag="po")
            for h in range(HPC):
                for nb in range(4):
                    j0 = half * 2048 + nb * 512
                    nc.tensor.matmul(po[:, nb * 512:(nb + 1) * 512],
                                     woin[:, h * B:(h + 1) * B], wo_sb[:, h, j0:j0 + 512],
                                     start=(h == 0), stop=(h == HPC - 1))
            osb = const.tile([B, HID // 2], f32, tag=f"osb{half}")
            nc.scalar.copy(osb[:], po[:])
            nc.sync.dma_start(out=out[:, half * 2048:(half + 1) * 2048], in_=osb[:])

    nc.compile()
    return nc


def _host_prep(inputs):
    hs = np.asarray(inputs["hidden_states"], np.float32)
    pos = np.asarray(inputs["position_ids"])
    inv = 1.0 / (THETA ** (np.arange(0, D, 2, dtype=np.float32) / D))
    fr = pos[:, 0].astype(np.float32)[:, None] * inv[None, :]
    emb = np.concatenate([fr, fr], axis=1)
    cos_b = np.cos(emb).astype(np.float32)
    sin_b = np.sin(emb).astype(np.float32)
    cosin = np.ascontiguousarray(
        np.concatenate([np.tile(cos_b, (1, HPC)), np.tile(sin_b, (1, HPC))], axis=1))
    # hidq [128, NCH*B] bf16
    hidq = np.ascontiguousarray(
        hs[:, 0, :].T.reshape(NCH, 128, B).transpose(1, 0, 2).reshape(128, NCH * B)
    ).astype(BF16)

    wq, wk, wv, wo = (np.asarray(inputs[k], np.float32) for k in ("wq", "wk", "wv", "wo"))
    kq_f = np.asarray(inputs["k_quant"])
    vq_f = np.asarray(inputs["v_quant"])
    ks_f = np.asarray(inputs["k_scale"], np.float32)
    km_f = np.asarray(inputs["k_mn"], np.float32)
    kf_f = np.asarray(inputs["k_full"], np.float32)
    kp_f = np.asarray(inputs["key_p"], np.float32)
    kqr_f = np.asarray(inputs["key_q"], np.float32)
    vs_f = np.asarray(inputs["v_scale"], np.float32)
    vm_f = np.asarray(inputs["v_mn"], np.float32)
    vf_f = np.asarray(inputs["v_full"], np.float32)
    vqv_f = np.asarray(inputs["value_q"], np.float32)
    vp_f = np.asarray(inputs["value_p"], np.float32)

    in_maps = []
    for core in range(NCORES):
        h0 = core * HPC
        sl = slice(h0 * D, (h0 + HPC) * D)
        hsl = slice(h0, h0 + HPC)

        wqkv = np.concatenate([wq[sl].T, wk[sl].T, wv[sl].T], axis=1)  # [4096, 1536]
        wqkv = wqkv.reshape(NCH, 128, 3 * HPC * D).transpose(1, 0, 2).reshape(128, -1)
        woc = wo[:, sl].T.reshape(HPC, 128, HID).transpose(1, 0, 2).reshape(128, -1)

        codes = np.empty((NP, 128, 2 * SQ), FP8)
        codes[:, :, 0:SQ] = kq_f[:, hsl].reshape(NP, 128, SQ).astype(FP8)
        # vcode: [B,HPC,SQ,D] -> [pair, p=s%128, c, d]
        codes[:, :, SQ:2 * SQ] = (vq_f[:, hsl].reshape(B, HPC, NCH, 128, D)
                                  .transpose(0, 1, 3, 2, 4).reshape(NP, 128, NCH * 128)
                                  .astype(FP8))

        kblob = np.zeros((128, NP, KBW), np.float32)
        kblob[:, :, 0:G] = ks_f[:, hsl].reshape(NP, 128, G).transpose(1, 0, 2)
        kblob[:, :, G:2 * G] = km_f[:, hsl].reshape(NP, 128, G).transpose(1, 0, 2)
        kblob[:, :, 128:132] = kqr_f[:, hsl].reshape(NP, 128, RANK).transpose(1, 0, 2)
        kblob[:, :, 132:132 + SF] = kf_f[:, hsl].reshape(NP, SF, 128).transpose(2, 0, 1)
        kblob[:, :, 196:196 + NCH * RANK] = (
            kp_f[:, hsl].reshape(B, HPC, NCH, 128, RANK)
            .transpose(3, 0, 1, 2, 4).reshape(128, NP, NCH * RANK))

        vblob = np.zeros((128, NP, VBW), np.float32)
        vblob[:, :, 0:NCH * FD] = (
            vs_f[:, hsl].reshape(B, HPC, NCH, 128, FD)
            .transpose(3, 0, 1, 2, 4).reshape(128, NP, NCH * FD))
        vblob[:, :, NCH * FD:NCH * FD + 4 * NCH] = (
            vqv_f[:, hsl].reshape(B, HPC, NCH, 128, RANK)
            .transpose(3, 0, 1, 4, 2).reshape(128, NP, 4 * NCH))
        vblob[:, :, NCH * FD + 4 * NCH:VBW] = (
            vm_f[:, hsl].reshape(B, HPC, NCH, 128, FD)
            .transpose(3, 0, 1, 4, 2).reshape(128, NP, 2 * NCH))

        vpm = np.zeros((6, NP, 128), np.float32)
        vpm[0:4] = vp_f[:, hsl].reshape(NP, 128, RANK).transpose(2, 0, 1)
        vpm[4, :, 0:64] = 1.0
        vpm[5, :, 64:128] = 1.0

        vfull = vf_f[:, hsl].reshape(NP, SF, 128).transpose(1, 0, 2)

        m = {
            "hidq": hidq, "cosin": cosin,
            "wqkv": np.ascontiguousarray(wqkv).astype(BF16),
            "woc": np.ascontiguousarray(woc).astype(BF16),
            "codes": codes,
            "kblob": kblob.reshape(128, NP * KBW).astype(BF16),
            "vblob": vblob.reshape(128, NP * VBW).astype(BF16),
            "vpm": vpm.reshape(6, NP * 128).astype(BF16),
            "vfull": np.ascontiguousarray(vfull).reshape(SF, NP * 128).astype(BF16),
        }
        in_maps.append(m)
    return in_maps


def kernel(**inputs):
    if "nc" not in _CACHE:
        _CACHE["nc"] = _build()
    nc = _CACHE["nc"]
    in_maps = _host_prep(inputs)
    res = run_bass_kernel_spmd(nc, in_maps, list(range(NCORES)),
                               trace=bool(os.environ.get("K_TRACE")))
    kernel.last = res
    total = np.zeros((B, HID), np.float32)
    for r in res.results:
        total += r["out"]
    return total.reshape(B, QL, HID)
